# revision 70
# baseline (speedup 1.0000x reference)
"""PrRoIPool2D (precise ROI pooling) Trainium2 kernel — 8-core SPMD.

Strategy ("fused banded sweep", v2):
  out[r,c,p,q] = sum_{h,w} F[b_r,c,h,w] * Iy[r,p,h] * Ix[r,q,w]
The (Iy ⊗ Ix) basis is banded: bin (r,p) touches only a ~4-row window of h.
Each core owns one feature batch.  Host packs a basis tensor B whose columns
are (r,p,q) output septets; for each 2-row h-chunk k the alive columns form
one contiguous interval [LO_k, HI_k).  The device runs one matmul per
(chunk, c-half, psum-bank-piece) with the features as stationary weights,
PSUM-accumulating straight into the final output columns.

v2 over v1:
  * Columns are assigned by GLOBAL sorted position of the (lo,hi) window
    across all cores (not per-core rank), so per-chunk alive intervals align
    across cores and the shared [LO_k,HI_k) union carries ~2.5x less waste.
  * K = 128 on the PE array with DRAM-zero pad rows on both operands
    (112-row matmuls measure ~1.4x slower per column, and garbage pad rows
    NaN-poison the product since NaN*0=NaN).
  * Output staged/stored as bf16 — halves output DMA (error budget is ample).
"""

import numpy as np
import ml_dtypes

POOLED = 7
SCALE = 0.5
N, C, H, W = 8, 256, 56, 56
NCORES = 8
CHUNK_H = 2
NCHUNK = H // CHUNK_H          # 28
KDIM = CHUNK_H * W             # 112 (payload K rows; DMAed)
KPAD = 128                     # PE-array K (pad rows zeroed on device)
BANK = 512                     # fp32 elements per PSUM bank
BF16 = ml_dtypes.bfloat16
SPLITS = (0, 7, 11, 14, 18, 22, 28)  # korder positions per input DMA split

_kernel_cache = {}
LAST_RESULTS = None            # BassKernelResults stash for test harnesses


def _tent_integral(start, end, n):
    i = np.arange(n, dtype=np.float64)
    a = np.clip(start[..., None] - i, -1.0, 1.0)
    b = np.clip(end[..., None] - i, -1.0, 1.0)

    def G(t):
        return np.where(t <= 0.0, 0.5 * (t + 1.0) ** 2, 1.0 - 0.5 * (1.0 - t) ** 2)

    return G(b) - G(a)


def _host_prep(features, rois):
    """Build per-core packed device inputs + unpack metadata."""
    R = rois.shape[0]
    batch = rois[:, 0].astype(np.int32)
    x1 = rois[:, 1].astype(np.float64) * SCALE
    y1 = rois[:, 2].astype(np.float64) * SCALE
    x2 = rois[:, 3].astype(np.float64) * SCALE
    y2 = rois[:, 4].astype(np.float64) * SCALE
    bw = (x2 - x1) / POOLED
    bh = (y2 - y1) / POOLED
    pw = np.arange(POOLED, dtype=np.float64)
    xs = x1[:, None] + pw * bw[:, None]
    ys = y1[:, None] + pw * bh[:, None]
    Ix = _tent_integral(xs, xs + bw[:, None], W)       # [R,7,W]
    Iy = _tent_integral(ys, ys + bh[:, None], H)       # [R,7,H]
    area = bw * bh
    scl = np.where(area > 0, 1.0 / np.maximum(area, 1e-12), 0.0)
    Iy_s = Iy * scl[:, None, None]

    core_rois = [np.nonzero(batch == c)[0] for c in range(NCORES)]
    Rmax = max(len(ix) for ix in core_rois)
    NSLOT = Rmax * POOLED
    COLS = NSLOT * POOLED
    NBANK = (COLS + BANK - 1) // BANK

    # real group windows per core, sorted by (lo, hi)
    groups = []                                        # [core][(lo,hi,rg,p)]
    for c in range(NCORES):
        wins = []
        for rg in core_rois[c]:
            for p in range(POOLED):
                nz = np.nonzero(Iy_s[rg, p] != 0)[0]
                lo, hi = (int(nz[0]), int(nz[-1])) if len(nz) else (0, 0)
                wins.append((lo, hi, int(rg), p))
        wins.sort(key=lambda t: (t[0], t[1]))
        groups.append(wins)

    # global-CDF slot assignment: the j-th window of core c gets a slot near
    # its global sorted position * NSLOT / G_tot, made strictly increasing.
    entries = [(lo, hi, c, j)
               for c in range(NCORES)
               for j, (lo, hi, _, _) in enumerate(groups[c])]
    entries.sort(key=lambda t: (t[0], t[1]))
    G_tot = len(entries)
    slots = [np.zeros(len(groups[c]), dtype=np.int64) for c in range(NCORES)]
    tgt = [np.zeros(len(groups[c])) for c in range(NCORES)]
    for r, (lo, hi, c, j) in enumerate(entries):
        tgt[c][j] = (r + 0.5) * NSLOT / G_tot
    for c in range(NCORES):
        n = len(groups[c])
        prev = -1
        for j in range(n):
            v = max(prev + 1, int(tgt[c][j]))
            slots[c][j] = v
            prev = v
        nxt = NSLOT
        for j in reversed(range(n)):
            v = min(int(slots[c][j]), nxt - 1)
            slots[c][j] = v
            nxt = v

    # per-chunk alive slot interval (min/max over cores)
    LO = np.full(NCHUNK, NSLOT, dtype=np.int64)
    HI = np.full(NCHUNK, -1, dtype=np.int64)
    for c in range(NCORES):
        for j, (lo, hi, rg, p) in enumerate(groups[c]):
            s = slots[c][j]
            for k in range(lo // CHUNK_H, hi // CHUNK_H + 1):
                LO[k] = min(LO[k], s)
                HI[k] = max(HI[k], s + 1)
    active = HI >= 0
    kact = [k for k in range(NCHUNK) if active[k]]
    # chain fix: every output column in [0, NSLOT) must be covered by some
    # chunk's interval (PSUM is read back in full), and consecutive intervals
    # must not leave gaps.
    running = 0
    for k in kact:
        if LO[k] > running:
            LO[k] = running
        running = max(running, HI[k])
    if running < NSLOT:
        HI[kact[-1]] = NSLOT
    LOc, HIc = LO * POOLED, HI * POOLED

    # chunk processing order (ascending h; reorderings measured worse on HW)
    korder = [k for k in range(NCHUNK) if active[k]]
    offs = {}                                  # B block start per chunk
    P = 0
    for k in korder:
        offs[k] = P
        P += int(HIc[k] - LOc[k])
    NB = P

    # pack B per core: B[(dh,w), packed_col]
    B = np.zeros((NCORES, KDIM, NB), dtype=np.float32)
    IxT = Ix.transpose(0, 2, 1)                        # [R, W, 7]
    for c in range(NCORES):
        for j, (lo, hi, rg, p) in enumerate(groups[c]):
            s = int(slots[c][j])
            for k in range(lo // CHUNK_H, hi // CHUNK_H + 1):
                cb = int(offs[k]) + s * POOLED - int(LOc[k])
                for dh in range(CHUNK_H):
                    h = CHUNK_H * k + dh
                    if lo <= h <= hi:
                        B[c, dh * W:(dh + 1) * W, cb:cb + POOLED] = (
                            Iy_s[rg, p, h] * IxT[rg]
                        )
    # Ship the 16 PE pad rows as DRAM zeros: on-device zeroing measured
    # slower (112-row DMAs lose ~40% per-byte ring efficiency and the
    # memzero deps block the sync sequencer's triggers).
    B = np.pad(B, ((0, 0), (0, KPAD - KDIM), (0, 0))).astype(BF16)

    # features per core, chunk-major transposed: FT[(dh,w), k*C + cc]
    f = features.astype(np.float32)                    # [N,C,H,W]
    ft = f.reshape(N, C, NCHUNK, CHUNK_H, W).transpose(0, 3, 4, 2, 1)
    FT = np.pad(ft.reshape(N, KDIM, NCHUNK * C),
                ((0, 0), (0, KPAD - KDIM), (0, 0))).astype(BF16)

    # merged input image: per split s, [ft cols | B cols] of its chunks,
    # contiguous, so one DMA per split moves one fat descriptor per
    # partition row.  Splits partition korder (widest chunks first).
    bounds = sorted(set(min(b, len(korder)) for b in SPLITS))
    pieces, ft_off, b_off, split_cols, P = [], {}, {}, [], 0
    for s in range(len(bounds) - 1):
        ks = korder[bounds[s]:bounds[s + 1]]
        a0 = P
        for k in ks:
            pieces.append(FT[:, :, k * C:(k + 1) * C])
            ft_off[k] = P
            P += C
        for k in ks:
            w = int(HIc[k] - LOc[k])
            pieces.append(B[:, :, offs[k]:offs[k] + w])
            b_off[k] = P
            P += w
        split_cols.append((a0, P))
    IN = np.concatenate(pieces, axis=2)
    assert IN.shape == (NCORES, KPAD, P) and P == len(korder) * C + NB

    return dict(IN=IN, korder=korder, LOc=LOc.astype(int), HIc=HIc.astype(int),
                groups=groups, slots=slots,
                ft_off=ft_off, b_off=b_off, split_cols=split_cols, TOT=P,
                Rmax=Rmax, COLS=COLS, NBANK=NBANK, NB=NB, R=R)


def _build_bass(shape_key):
    """Build + compile the SPMD Bass program for given packing metadata."""
    (NB, COLS, NBANK, LOc, HIc, korder_t, ft_off_t, b_off_t,
     split_cols_t, TOT) = shape_key
    LOc, HIc, korder = list(LOc), list(HIc), list(korder_t)
    ft_off, b_off = dict(ft_off_t), dict(b_off_t)
    split_cols = list(split_cols_t)

    import concourse.bass as bass  # noqa: F401
    import concourse.tile as tile
    from concourse import bacc, mybir

    nc = bacc.Bacc("TRN2", target_bir_lowering=False, debug=False,
                   enable_asserts=False, num_devices=NCORES)
    bf = mybir.dt.bfloat16
    f32 = mybir.dt.float32
    in_ap = nc.dram_tensor("inp", [KPAD, TOT], bf, kind="ExternalInput").ap()
    out_ap = nc.dram_tensor("out", [C, COLS], bf, kind="ExternalOutput").ap()

    # last chunk (in processing order) touching each bank → stop flag;
    # matmuls may not cross a PSUM bank boundary (invalid ISA), so pieces
    # split per bank.
    last_k = {}
    for k in korder:
        for bk in range(LOc[k] // BANK, (HIc[k] - 1) // BANK + 1):
            last_k[bk] = k

    with tile.TileContext(nc) as tc:
        with (
            tc.tile_pool(name="inp", bufs=1) as inp,
            tc.tile_pool(name="pp", bufs=8, space="PSUM") as pp,
            tc.tile_pool(name="op", bufs=2) as op,
        ):
            in_sb = inp.tile([KPAD, TOT], bf)
            # one fat DMA per split (ft+B merged columns): one ~8KB
            # descriptor per partition row, all triggered from sync.
            for a, bnd in split_cols:
                nc.sync.dma_start(in_sb[:, a:bnd], in_ap[:, a:bnd])

            ptiles = [[pp.tile([128, BANK], f32, tag="bank", name=f"pt{m}_{i}")
                       for i in range(NBANK)] for m in range(2)]
            out_sb = [op.tile([128, COLS], bf, name=f"os{m}") for m in range(2)]
            # cols written so far per bank (has_written high-water mark);
            # -1 = bank untouched.  Per-element has_written semantics make
            # first-write overwrite, later writes add.  Both c-halves run
            # inside the chunk loop so the tensor engine tracks the B stream
            # instead of queueing half the work behind the last input split.
            whi = [[-1] * NBANK for _ in range(2)]
            for k in korder:
                lo, hi = LOc[k], HIc[k]
                fo, bo = ft_off[k], b_off[k]
                for m in range(2):
                    lhsT = in_sb[:, fo + m * 128: fo + (m + 1) * 128]
                    for bk in range(lo // BANK, (hi - 1) // BANK + 1):
                        s = max(lo, bk * BANK)
                        e = min(hi, (bk + 1) * BANK)
                        nc.tensor.matmul(
                            ptiles[m][bk][:, s - bk * BANK: e - bk * BANK],
                            lhsT=lhsT,
                            rhs=in_sb[:, bo + s - lo: bo + e - lo],
                            start=whi[m][bk] < 0,
                            stop=k == last_k[bk],
                        )
                        whi[m][bk] = max(whi[m][bk], e)
            # evacuate each PSUM bank as it retires: half0 on vector, half1
            # on scalar (gpsimd cannot access PSUM).  Stores go out in two
            # pieces per half, with piece-1 triggers emitted BETWEEN the
            # copy groups: scalar executes in program order, so a trigger
            # emitted after the bank-3 copy would idle until the last
            # matmul; emitted here it streams during the input tail.
            # Copies: banks 0-2 split vector(m0)/scalar(m1); BOTH bank-3
            # copies on vector (it frees first after the last matmul, so the
            # final store's data is ready ~1us sooner than with scalar).
            # All store triggers on sync — idle after the input triggers —
            # so no trigger queues behind a copy on the same sequencer.
            cut = 2 * BANK
            for bk in range(NBANK):
                w = min(BANK, COLS - bk * BANK)
                for m in range(2):
                    dst = out_sb[m][:, bk * BANK: bk * BANK + w]
                    if m == 0 or bk == NBANK - 1:
                        nc.vector.tensor_copy(dst, ptiles[m][bk][:, :w])
                    else:
                        nc.scalar.copy(dst, ptiles[m][bk][:, :w])
                if bk == 1:
                    for m in range(2):
                        nc.sync.dma_start(
                            out_ap[m * 128:(m + 1) * 128, :cut],
                            out_sb[m][:, :cut])
            # final pieces: one trigger on sync, one on scalar (free after
            # its bank-2 copy) so the two ~0.6us trigger setups overlap.
            nc.sync.dma_start(out_ap[:128, cut:COLS], out_sb[0][:, cut:COLS])
            nc.scalar.dma_start(out_ap[128:, cut:COLS], out_sb[1][:, cut:COLS])

    nc.compile()
    return nc


def _ensure_ntff_hook():
    """Some images lack antenv.axon_hooks; recreate it so a BASS_TRACE=1
    environment degrades to (or succeeds at) profiling instead of crashing."""
    import sys
    try:
        import antenv.axon_hooks  # noqa: F401
        return
    except ImportError:
        pass
    try:
        import types
        import antenv
        mod = types.ModuleType("antenv.axon_hooks")
        _hook = [None]
        mod.set_axon_ntff_profile_hook = lambda h: _hook.__setitem__(0, h)
        mod.get_axon_ntff_profile_hook = lambda: _hook[0]
        sys.modules["antenv.axon_hooks"] = mod
        antenv.axon_hooks = mod
        from trn_agent_boot.trn_boot import _ntff_profile_via_ctypes
        mod.set_axon_ntff_profile_hook(
            _ntff_profile_via_ctypes("/opt/axon/libaxon_pjrt.so"))
    except Exception:
        pass


def _unpack(res, hp):
    """out_core[c_chan, col(slot,q)] -> final[r, c_chan, p, q]"""
    final = np.zeros((hp["R"], C, POOLED, POOLED), dtype=np.float32)
    for c in range(NCORES):
        out = np.asarray(res.results[c]["out"]).astype(np.float32)  # [C, COLS]
        gs = hp["groups"][c]
        if not gs:
            continue
        rgs = np.array([g[2] for g in gs])
        ps = np.array([g[3] for g in gs])
        sl = np.asarray(hp["slots"][c], dtype=np.int64)
        cols = out.reshape(C, -1, POOLED)[:, sl, :]     # [C, ngrp, 7]
        final[rgs, :, ps, :] = cols.transpose(1, 0, 2)
    return final


def kernel(features, rois):
    global LAST_RESULTS
    _ensure_ntff_hook()
    from concourse import bass_utils

    features = np.asarray(features, dtype=np.float32)
    rois = np.asarray(rois, dtype=np.float32)
    hp = _host_prep(features, rois)

    shape_key = (hp["NB"], hp["COLS"], hp["NBANK"],
                 tuple(hp["LOc"]), tuple(hp["HIc"]),
                 tuple(hp["korder"]),
                 tuple(sorted(hp["ft_off"].items())),
                 tuple(sorted(hp["b_off"].items())),
                 tuple(hp["split_cols"]),
                 int(hp["TOT"]))
    nc = _kernel_cache.get(shape_key)
    if nc is None:
        nc = _build_bass(shape_key)
        _kernel_cache[shape_key] = nc

    in_maps = [{"inp": np.ascontiguousarray(hp["IN"][c])}
               for c in range(NCORES)]
    # flaky-device insurance: a wedged core occasionally returns NaN — rerun.
    final = None
    for attempt in range(3):
        res = bass_utils.run_bass_kernel_spmd(nc, in_maps,
                                              core_ids=list(range(NCORES)))
        LAST_RESULTS = res
        final = _unpack(res, hp)
        if np.isfinite(final).all():
            break
    return final


# revision 71
# speedup vs baseline: 1.0340x; 1.0340x over previous
"""PrRoIPool2D (precise ROI pooling) Trainium2 kernel — 8-core SPMD.

Strategy ("fused banded sweep", v2):
  out[r,c,p,q] = sum_{h,w} F[b_r,c,h,w] * Iy[r,p,h] * Ix[r,q,w]
The (Iy ⊗ Ix) basis is banded: bin (r,p) touches only a ~4-row window of h.
Each core owns one feature batch.  Host packs a basis tensor B whose columns
are (r,p,q) output septets; for each 2-row h-chunk k the alive columns form
one contiguous interval [LO_k, HI_k).  The device runs one matmul per
(chunk, c-half, psum-bank-piece) with the features as stationary weights,
PSUM-accumulating straight into the final output columns.

v2 over v1:
  * Columns are assigned by GLOBAL sorted position of the (lo,hi) window
    across all cores (not per-core rank), so per-chunk alive intervals align
    across cores and the shared [LO_k,HI_k) union carries ~2.5x less waste.
  * K = 128 on the PE array with DRAM-zero pad rows on both operands
    (112-row matmuls measure ~1.4x slower per column, and garbage pad rows
    NaN-poison the product since NaN*0=NaN).
  * Output staged/stored as bf16 — halves output DMA (error budget is ample).
"""

import numpy as np
import ml_dtypes

POOLED = 7
SCALE = 0.5
N, C, H, W = 8, 256, 56, 56
NCORES = 8
CHUNK_H = 2
NCHUNK = H // CHUNK_H          # 28
KDIM = CHUNK_H * W             # 112 (payload K rows; DMAed)
KPAD = 128                     # PE-array K (pad rows zeroed on device)
BANK = 512                     # fp32 elements per PSUM bank
BF16 = ml_dtypes.bfloat16
SPLITS = (0, 7, 11, 14, 18, 22, 28)  # korder positions per input DMA split

_kernel_cache = {}
LAST_RESULTS = None            # BassKernelResults stash for test harnesses


def _tent_integral(start, end, n):
    i = np.arange(n, dtype=np.float64)
    a = np.clip(start[..., None] - i, -1.0, 1.0)
    b = np.clip(end[..., None] - i, -1.0, 1.0)

    def G(t):
        return np.where(t <= 0.0, 0.5 * (t + 1.0) ** 2, 1.0 - 0.5 * (1.0 - t) ** 2)

    return G(b) - G(a)


def _host_prep(features, rois):
    """Build per-core packed device inputs + unpack metadata."""
    R = rois.shape[0]
    batch = rois[:, 0].astype(np.int32)
    x1 = rois[:, 1].astype(np.float64) * SCALE
    y1 = rois[:, 2].astype(np.float64) * SCALE
    x2 = rois[:, 3].astype(np.float64) * SCALE
    y2 = rois[:, 4].astype(np.float64) * SCALE
    bw = (x2 - x1) / POOLED
    bh = (y2 - y1) / POOLED
    pw = np.arange(POOLED, dtype=np.float64)
    xs = x1[:, None] + pw * bw[:, None]
    ys = y1[:, None] + pw * bh[:, None]
    Ix = _tent_integral(xs, xs + bw[:, None], W)       # [R,7,W]
    Iy = _tent_integral(ys, ys + bh[:, None], H)       # [R,7,H]
    area = bw * bh
    scl = np.where(area > 0, 1.0 / np.maximum(area, 1e-12), 0.0)
    Iy_s = Iy * scl[:, None, None]

    core_rois = [np.nonzero(batch == c)[0] for c in range(NCORES)]
    Rmax = max(len(ix) for ix in core_rois)
    NSLOT = Rmax * POOLED
    COLS = NSLOT * POOLED
    NBANK = (COLS + BANK - 1) // BANK

    # real group windows per core, sorted by (lo, hi)
    groups = []                                        # [core][(lo,hi,rg,p)]
    for c in range(NCORES):
        wins = []
        for rg in core_rois[c]:
            for p in range(POOLED):
                nz = np.nonzero(Iy_s[rg, p] != 0)[0]
                lo, hi = (int(nz[0]), int(nz[-1])) if len(nz) else (0, 0)
                wins.append((lo, hi, int(rg), p))
        wins.sort(key=lambda t: (t[0], t[1]))
        groups.append(wins)

    # global-CDF slot assignment: the j-th window of core c gets a slot near
    # its global sorted position * NSLOT / G_tot, made strictly increasing.
    entries = [(lo, hi, c, j)
               for c in range(NCORES)
               for j, (lo, hi, _, _) in enumerate(groups[c])]
    entries.sort(key=lambda t: (t[0], t[1]))
    G_tot = len(entries)
    slots = [np.zeros(len(groups[c]), dtype=np.int64) for c in range(NCORES)]
    tgt = [np.zeros(len(groups[c])) for c in range(NCORES)]
    for r, (lo, hi, c, j) in enumerate(entries):
        tgt[c][j] = (r + 0.5) * NSLOT / G_tot
    for c in range(NCORES):
        n = len(groups[c])
        prev = -1
        for j in range(n):
            v = max(prev + 1, int(tgt[c][j]))
            slots[c][j] = v
            prev = v
        nxt = NSLOT
        for j in reversed(range(n)):
            v = min(int(slots[c][j]), nxt - 1)
            slots[c][j] = v
            nxt = v

    # per-chunk alive slot interval (min/max over cores)
    LO = np.full(NCHUNK, NSLOT, dtype=np.int64)
    HI = np.full(NCHUNK, -1, dtype=np.int64)
    for c in range(NCORES):
        for j, (lo, hi, rg, p) in enumerate(groups[c]):
            s = slots[c][j]
            for k in range(lo // CHUNK_H, hi // CHUNK_H + 1):
                LO[k] = min(LO[k], s)
                HI[k] = max(HI[k], s + 1)
    active = HI >= 0
    kact = [k for k in range(NCHUNK) if active[k]]
    # chain fix: every output column in [0, NSLOT) must be covered by some
    # chunk's interval (PSUM is read back in full), and consecutive intervals
    # must not leave gaps.
    running = 0
    for k in kact:
        if LO[k] > running:
            LO[k] = running
        running = max(running, HI[k])
    if running < NSLOT:
        HI[kact[-1]] = NSLOT
    LOc, HIc = LO * POOLED, HI * POOLED

    # chunk processing order (ascending h; reorderings measured worse on HW)
    korder = [k for k in range(NCHUNK) if active[k]]
    offs = {}                                  # B block start per chunk
    P = 0
    for k in korder:
        offs[k] = P
        P += int(HIc[k] - LOc[k])
    NB = P

    # pack B per core: B[(dh,w), packed_col]
    B = np.zeros((NCORES, KDIM, NB), dtype=np.float32)
    IxT = Ix.transpose(0, 2, 1)                        # [R, W, 7]
    for c in range(NCORES):
        for j, (lo, hi, rg, p) in enumerate(groups[c]):
            s = int(slots[c][j])
            for k in range(lo // CHUNK_H, hi // CHUNK_H + 1):
                cb = int(offs[k]) + s * POOLED - int(LOc[k])
                for dh in range(CHUNK_H):
                    h = CHUNK_H * k + dh
                    if lo <= h <= hi:
                        B[c, dh * W:(dh + 1) * W, cb:cb + POOLED] = (
                            Iy_s[rg, p, h] * IxT[rg]
                        )
    # Ship the 16 PE pad rows as DRAM zeros: on-device zeroing measured
    # slower (112-row DMAs lose ~40% per-byte ring efficiency and the
    # memzero deps block the sync sequencer's triggers).
    B = np.pad(B, ((0, 0), (0, KPAD - KDIM), (0, 0))).astype(BF16)

    # features per core, chunk-major transposed: FT[(dh,w), k*C + cc]
    f = features.astype(np.float32)                    # [N,C,H,W]
    ft = f.reshape(N, C, NCHUNK, CHUNK_H, W).transpose(0, 3, 4, 2, 1)
    FT = np.pad(ft.reshape(N, KDIM, NCHUNK * C),
                ((0, 0), (0, KPAD - KDIM), (0, 0))).astype(BF16)

    # merged input image: per split s, [ft cols | B cols] of its chunks,
    # contiguous, so one DMA per split moves one fat descriptor per
    # partition row.  Splits partition korder (widest chunks first).
    bounds = sorted(set(min(b, len(korder)) for b in SPLITS))
    pieces, ft_off, b_off, split_cols, P = [], {}, {}, [], 0
    for s in range(len(bounds) - 1):
        ks = korder[bounds[s]:bounds[s + 1]]
        a0 = P
        for k in ks:
            pieces.append(FT[:, :, k * C:(k + 1) * C])
            ft_off[k] = P
            P += C
        for k in ks:
            w = int(HIc[k] - LOc[k])
            pieces.append(B[:, :, offs[k]:offs[k] + w])
            b_off[k] = P
            P += w
        split_cols.append((a0, P))
    IN = np.concatenate(pieces, axis=2)
    assert IN.shape == (NCORES, KPAD, P) and P == len(korder) * C + NB

    return dict(IN=IN, korder=korder, LOc=LOc.astype(int), HIc=HIc.astype(int),
                groups=groups, slots=slots,
                ft_off=ft_off, b_off=b_off, split_cols=split_cols, TOT=P,
                Rmax=Rmax, COLS=COLS, NBANK=NBANK, NB=NB, R=R)


def _build_bass(shape_key):
    """Build + compile the SPMD Bass program for given packing metadata."""
    (NB, COLS, NBANK, LOc, HIc, korder_t, ft_off_t, b_off_t,
     split_cols_t, TOT) = shape_key
    LOc, HIc, korder = list(LOc), list(HIc), list(korder_t)
    ft_off, b_off = dict(ft_off_t), dict(b_off_t)
    split_cols = list(split_cols_t)

    import concourse.bass as bass  # noqa: F401
    import concourse.tile as tile
    from concourse import bacc, mybir

    nc = bacc.Bacc("TRN2", target_bir_lowering=False, debug=False,
                   enable_asserts=False, num_devices=NCORES)
    bf = mybir.dt.bfloat16
    f32 = mybir.dt.float32
    in_ap = nc.dram_tensor("inp", [KPAD, TOT], bf, kind="ExternalInput").ap()
    out_ap = nc.dram_tensor("out", [C, COLS], bf, kind="ExternalOutput").ap()

    # last chunk (in processing order) touching each bank → stop flag;
    # matmuls may not cross a PSUM bank boundary (invalid ISA), so pieces
    # split per bank.
    last_k = {}
    for k in korder:
        for bk in range(LOc[k] // BANK, (HIc[k] - 1) // BANK + 1):
            last_k[bk] = k

    with tile.TileContext(nc) as tc:
        with (
            tc.tile_pool(name="inp", bufs=1) as inp,
            tc.tile_pool(name="pp", bufs=8, space="PSUM") as pp,
            tc.tile_pool(name="op", bufs=2) as op,
        ):
            in_sb = inp.tile([KPAD, TOT], bf)
            # one fat DMA per split (ft+B merged columns): one ~8KB
            # descriptor per partition row, all triggered from sync.
            for a, bnd in split_cols:
                nc.sync.dma_start(in_sb[:, a:bnd], in_ap[:, a:bnd])

            ptiles = [[pp.tile([128, BANK], f32, tag="bank", name=f"pt{m}_{i}")
                       for i in range(NBANK)] for m in range(2)]
            out_sb = [op.tile([128, COLS], bf, name=f"os{m}") for m in range(2)]
            # cols written so far per bank (has_written high-water mark);
            # -1 = bank untouched.  Per-element has_written semantics make
            # first-write overwrite, later writes add.  Both c-halves run
            # inside the chunk loop so the tensor engine tracks the B stream
            # instead of queueing half the work behind the last input split.
            whi = [[-1] * NBANK for _ in range(2)]
            for k in korder:
                lo, hi = LOc[k], HIc[k]
                fo, bo = ft_off[k], b_off[k]
                for m in range(2):
                    lhsT = in_sb[:, fo + m * 128: fo + (m + 1) * 128]
                    for bk in range(lo // BANK, (hi - 1) // BANK + 1):
                        s = max(lo, bk * BANK)
                        e = min(hi, (bk + 1) * BANK)
                        nc.tensor.matmul(
                            ptiles[m][bk][:, s - bk * BANK: e - bk * BANK],
                            lhsT=lhsT,
                            rhs=in_sb[:, bo + s - lo: bo + e - lo],
                            start=whi[m][bk] < 0,
                            stop=k == last_k[bk],
                        )
                        whi[m][bk] = max(whi[m][bk], e)
            # evacuate each PSUM bank as it retires: half0 on vector, half1
            # on scalar (gpsimd cannot access PSUM).  Stores go out in two
            # pieces per half, with piece-1 triggers emitted BETWEEN the
            # copy groups: scalar executes in program order, so a trigger
            # emitted after the bank-3 copy would idle until the last
            # matmul; emitted here it streams during the input tail.
            # Copies: banks 0-2 split vector(m0)/scalar(m1); BOTH bank-3
            # copies on vector (it frees first after the last matmul, so the
            # final store's data is ready ~1us sooner than with scalar).
            # All store triggers on sync — idle after the input triggers —
            # so no trigger queues behind a copy on the same sequencer.
            cut = 2 * BANK
            for bk in range(NBANK):
                w = min(BANK, COLS - bk * BANK)
                for m in range(2):
                    dst = out_sb[m][:, bk * BANK: bk * BANK + w]
                    if m == 0 or bk == NBANK - 1:
                        nc.vector.tensor_copy(dst, ptiles[m][bk][:, :w])
                    else:
                        nc.scalar.copy(dst, ptiles[m][bk][:, :w])
                if bk == 1:
                    for m in range(2):
                        nc.sync.dma_start(
                            out_ap[m * 128:(m + 1) * 128, :cut],
                            out_sb[m][:, :cut])
            for m in range(2):
                nc.sync.dma_start(out_ap[m * 128:(m + 1) * 128, cut:COLS],
                                  out_sb[m][:, cut:COLS])

    nc.compile()
    return nc


def _ensure_ntff_hook():
    """Some images lack antenv.axon_hooks; recreate it so a BASS_TRACE=1
    environment degrades to (or succeeds at) profiling instead of crashing."""
    import sys
    try:
        import antenv.axon_hooks  # noqa: F401
        return
    except ImportError:
        pass
    try:
        import types
        import antenv
        mod = types.ModuleType("antenv.axon_hooks")
        _hook = [None]
        mod.set_axon_ntff_profile_hook = lambda h: _hook.__setitem__(0, h)
        mod.get_axon_ntff_profile_hook = lambda: _hook[0]
        sys.modules["antenv.axon_hooks"] = mod
        antenv.axon_hooks = mod
        from trn_agent_boot.trn_boot import _ntff_profile_via_ctypes
        mod.set_axon_ntff_profile_hook(
            _ntff_profile_via_ctypes("/opt/axon/libaxon_pjrt.so"))
    except Exception:
        pass


def _unpack(res, hp):
    """out_core[c_chan, col(slot,q)] -> final[r, c_chan, p, q]"""
    final = np.zeros((hp["R"], C, POOLED, POOLED), dtype=np.float32)
    for c in range(NCORES):
        out = np.asarray(res.results[c]["out"]).astype(np.float32)  # [C, COLS]
        gs = hp["groups"][c]
        if not gs:
            continue
        rgs = np.array([g[2] for g in gs])
        ps = np.array([g[3] for g in gs])
        sl = np.asarray(hp["slots"][c], dtype=np.int64)
        cols = out.reshape(C, -1, POOLED)[:, sl, :]     # [C, ngrp, 7]
        final[rgs, :, ps, :] = cols.transpose(1, 0, 2)
    return final


def kernel(features, rois):
    global LAST_RESULTS
    _ensure_ntff_hook()
    from concourse import bass_utils

    features = np.asarray(features, dtype=np.float32)
    rois = np.asarray(rois, dtype=np.float32)
    hp = _host_prep(features, rois)

    shape_key = (hp["NB"], hp["COLS"], hp["NBANK"],
                 tuple(hp["LOc"]), tuple(hp["HIc"]),
                 tuple(hp["korder"]),
                 tuple(sorted(hp["ft_off"].items())),
                 tuple(sorted(hp["b_off"].items())),
                 tuple(hp["split_cols"]),
                 int(hp["TOT"]))
    nc = _kernel_cache.get(shape_key)
    if nc is None:
        nc = _build_bass(shape_key)
        _kernel_cache[shape_key] = nc

    in_maps = [{"inp": np.ascontiguousarray(hp["IN"][c])}
               for c in range(NCORES)]
    # flaky-device insurance: a wedged core occasionally returns NaN — rerun.
    final = None
    for attempt in range(3):
        res = bass_utils.run_bass_kernel_spmd(nc, in_maps,
                                              core_ids=list(range(NCORES)))
        LAST_RESULTS = res
        final = _unpack(res, hp)
        if np.isfinite(final).all():
            break
    return final


# revision 75
# speedup vs baseline: 1.0666x; 1.0316x over previous
"""PrRoIPool2D (precise ROI pooling) Trainium2 kernel — 8-core SPMD.

Strategy ("fused banded sweep", v2):
  out[r,c,p,q] = sum_{h,w} F[b_r,c,h,w] * Iy[r,p,h] * Ix[r,q,w]
The (Iy ⊗ Ix) basis is banded: bin (r,p) touches only a ~4-row window of h.
Each core owns one feature batch.  Host packs a basis tensor B whose columns
are (r,p,q) output septets; for each 2-row h-chunk k the alive columns form
one contiguous interval [LO_k, HI_k).  The device runs one matmul per
(chunk, c-half, psum-bank-piece) with the features as stationary weights,
PSUM-accumulating straight into the final output columns.

v2 over v1:
  * Columns are assigned by GLOBAL sorted position of the (lo,hi) window
    across all cores (not per-core rank), so per-chunk alive intervals align
    across cores and the shared [LO_k,HI_k) union carries ~2.5x less waste.
  * K = 128 on the PE array with DRAM-zero pad rows on both operands
    (112-row matmuls measure ~1.4x slower per column, and garbage pad rows
    NaN-poison the product since NaN*0=NaN).
  * Output staged/stored as bf16 — halves output DMA (error budget is ample).
"""

import numpy as np
import ml_dtypes

POOLED = 7
SCALE = 0.5
N, C, H, W = 8, 256, 56, 56
NCORES = 8
CHUNK_H = 2
NCHUNK = H // CHUNK_H          # 28
KDIM = CHUNK_H * W             # 112 (payload K rows; DMAed)
KPAD = 128                     # PE-array K (pad rows zeroed on device)
BANK = 512                     # fp32 elements per PSUM bank
BF16 = ml_dtypes.bfloat16
SPLITS = (0, 7, 11, 14, 18, 22, 28)  # korder positions per input DMA split

_kernel_cache = {}
LAST_RESULTS = None            # BassKernelResults stash for test harnesses


def _tent_integral(start, end, n):
    i = np.arange(n, dtype=np.float64)
    a = np.clip(start[..., None] - i, -1.0, 1.0)
    b = np.clip(end[..., None] - i, -1.0, 1.0)

    def G(t):
        return np.where(t <= 0.0, 0.5 * (t + 1.0) ** 2, 1.0 - 0.5 * (1.0 - t) ** 2)

    return G(b) - G(a)


def _host_prep(features, rois):
    """Build per-core packed device inputs + unpack metadata."""
    R = rois.shape[0]
    batch = rois[:, 0].astype(np.int32)
    x1 = rois[:, 1].astype(np.float64) * SCALE
    y1 = rois[:, 2].astype(np.float64) * SCALE
    x2 = rois[:, 3].astype(np.float64) * SCALE
    y2 = rois[:, 4].astype(np.float64) * SCALE
    bw = (x2 - x1) / POOLED
    bh = (y2 - y1) / POOLED
    pw = np.arange(POOLED, dtype=np.float64)
    xs = x1[:, None] + pw * bw[:, None]
    ys = y1[:, None] + pw * bh[:, None]
    Ix = _tent_integral(xs, xs + bw[:, None], W)       # [R,7,W]
    Iy = _tent_integral(ys, ys + bh[:, None], H)       # [R,7,H]
    area = bw * bh
    scl = np.where(area > 0, 1.0 / np.maximum(area, 1e-12), 0.0)
    Iy_s = Iy * scl[:, None, None]

    core_rois = [np.nonzero(batch == c)[0] for c in range(NCORES)]
    Rmax = max(len(ix) for ix in core_rois)
    NSLOT = Rmax * POOLED
    COLS = NSLOT * POOLED
    NBANK = (COLS + BANK - 1) // BANK

    # real group windows per core, sorted by (lo, hi)
    groups = []                                        # [core][(lo,hi,rg,p)]
    for c in range(NCORES):
        wins = []
        for rg in core_rois[c]:
            for p in range(POOLED):
                nz = np.nonzero(Iy_s[rg, p] != 0)[0]
                lo, hi = (int(nz[0]), int(nz[-1])) if len(nz) else (0, 0)
                wins.append((lo, hi, int(rg), p))
        wins.sort(key=lambda t: (t[0], t[1]))
        groups.append(wins)

    # global-CDF slot assignment: the j-th window of core c gets a slot near
    # its global sorted position * NSLOT / G_tot, made strictly increasing.
    entries = [(lo, hi, c, j)
               for c in range(NCORES)
               for j, (lo, hi, _, _) in enumerate(groups[c])]
    entries.sort(key=lambda t: (t[0], t[1]))
    G_tot = len(entries)
    slots = [np.zeros(len(groups[c]), dtype=np.int64) for c in range(NCORES)]
    tgt = [np.zeros(len(groups[c])) for c in range(NCORES)]
    for r, (lo, hi, c, j) in enumerate(entries):
        tgt[c][j] = (r + 0.5) * NSLOT / G_tot
    for c in range(NCORES):
        n = len(groups[c])
        prev = -1
        for j in range(n):
            v = max(prev + 1, int(tgt[c][j]))
            slots[c][j] = v
            prev = v
        nxt = NSLOT
        for j in reversed(range(n)):
            v = min(int(slots[c][j]), nxt - 1)
            slots[c][j] = v
            nxt = v

    # per-chunk alive slot interval (min/max over cores)
    LO = np.full(NCHUNK, NSLOT, dtype=np.int64)
    HI = np.full(NCHUNK, -1, dtype=np.int64)
    for c in range(NCORES):
        for j, (lo, hi, rg, p) in enumerate(groups[c]):
            s = slots[c][j]
            for k in range(lo // CHUNK_H, hi // CHUNK_H + 1):
                LO[k] = min(LO[k], s)
                HI[k] = max(HI[k], s + 1)
    active = HI >= 0
    kact = [k for k in range(NCHUNK) if active[k]]
    # chain fix: every output column in [0, NSLOT) must be covered by some
    # chunk's interval (PSUM is read back in full), and consecutive intervals
    # must not leave gaps.
    running = 0
    for k in kact:
        if LO[k] > running:
            LO[k] = running
        running = max(running, HI[k])
    if running < NSLOT:
        HI[kact[-1]] = NSLOT
    LOc, HIc = LO * POOLED, HI * POOLED

    # chunk processing order (ascending h; reorderings measured worse on HW)
    korder = [k for k in range(NCHUNK) if active[k]]
    offs = {}                                  # B block start per chunk
    P = 0
    for k in korder:
        offs[k] = P
        P += int(HIc[k] - LOc[k])
    NB = P

    # pack B per core: B[(dh,w), packed_col]
    B = np.zeros((NCORES, KDIM, NB), dtype=np.float32)
    IxT = Ix.transpose(0, 2, 1)                        # [R, W, 7]
    for c in range(NCORES):
        for j, (lo, hi, rg, p) in enumerate(groups[c]):
            s = int(slots[c][j])
            for k in range(lo // CHUNK_H, hi // CHUNK_H + 1):
                cb = int(offs[k]) + s * POOLED - int(LOc[k])
                for dh in range(CHUNK_H):
                    h = CHUNK_H * k + dh
                    if lo <= h <= hi:
                        B[c, dh * W:(dh + 1) * W, cb:cb + POOLED] = (
                            Iy_s[rg, p, h] * IxT[rg]
                        )
    # Ship the 16 PE pad rows as DRAM zeros: on-device zeroing measured
    # slower (112-row DMAs lose ~40% per-byte ring efficiency and the
    # memzero deps block the sync sequencer's triggers).
    B = np.pad(B, ((0, 0), (0, KPAD - KDIM), (0, 0))).astype(BF16)

    # features per core, chunk-major transposed: FT[(dh,w), k*C + cc]
    f = features.astype(np.float32)                    # [N,C,H,W]
    ft = f.reshape(N, C, NCHUNK, CHUNK_H, W).transpose(0, 3, 4, 2, 1)
    FT = np.pad(ft.reshape(N, KDIM, NCHUNK * C),
                ((0, 0), (0, KPAD - KDIM), (0, 0))).astype(BF16)

    # merged input image: per split s, [ft cols | B cols] of its chunks,
    # contiguous, so one DMA per split moves one fat descriptor per
    # partition row.  Splits partition korder (widest chunks first).
    bounds = sorted(set(min(b, len(korder)) for b in SPLITS))
    pieces, ft_off, b_off, split_cols, P = [], {}, {}, [], 0
    for s in range(len(bounds) - 1):
        ks = korder[bounds[s]:bounds[s + 1]]
        a0 = P
        for k in ks:
            pieces.append(FT[:, :, k * C:(k + 1) * C])
            ft_off[k] = P
            P += C
        for k in ks:
            w = int(HIc[k] - LOc[k])
            pieces.append(B[:, :, offs[k]:offs[k] + w])
            b_off[k] = P
            P += w
        split_cols.append((a0, P))
    IN = np.concatenate(pieces, axis=2)
    assert IN.shape == (NCORES, KPAD, P) and P == len(korder) * C + NB

    return dict(IN=IN, korder=korder, LOc=LOc.astype(int), HIc=HIc.astype(int),
                groups=groups, slots=slots,
                ft_off=ft_off, b_off=b_off, split_cols=split_cols, TOT=P,
                Rmax=Rmax, COLS=COLS, NBANK=NBANK, NB=NB, R=R)


def _build_bass(shape_key):
    """Build + compile the SPMD Bass program for given packing metadata."""
    (NB, COLS, NBANK, LOc, HIc, korder_t, ft_off_t, b_off_t,
     split_cols_t, TOT) = shape_key
    LOc, HIc, korder = list(LOc), list(HIc), list(korder_t)
    ft_off, b_off = dict(ft_off_t), dict(b_off_t)
    split_cols = list(split_cols_t)

    import concourse.bass as bass  # noqa: F401
    import concourse.tile as tile
    from concourse import bacc, mybir

    nc = bacc.Bacc("TRN2", target_bir_lowering=False, debug=False,
                   enable_asserts=False, num_devices=NCORES)
    bf = mybir.dt.bfloat16
    f32 = mybir.dt.float32
    in_ap = nc.dram_tensor("inp", [KPAD, TOT], bf, kind="ExternalInput").ap()
    # output is bank-interleaved across the two c-halves so each store piece
    # is ONE contiguous DMA covering both halves: cols of bank bk, half m sit
    # at [2*512*bk + m*w_bk, ...); the host de-interleaves.
    out_ap = nc.dram_tensor("out", [128, 2 * COLS], bf,
                            kind="ExternalOutput").ap()

    # last chunk (in processing order) touching each bank → stop flag;
    # matmuls may not cross a PSUM bank boundary (invalid ISA), so pieces
    # split per bank.
    last_k = {}
    for k in korder:
        for bk in range(LOc[k] // BANK, (HIc[k] - 1) // BANK + 1):
            last_k[bk] = k

    with tile.TileContext(nc) as tc:
        with (
            tc.tile_pool(name="inp", bufs=1) as inp,
            tc.tile_pool(name="pp", bufs=8, space="PSUM") as pp,
            tc.tile_pool(name="op", bufs=2) as op,
        ):
            in_sb = inp.tile([KPAD, TOT], bf)
            # one fat DMA per split (ft+B merged columns): one ~8KB
            # descriptor per partition row, all triggered from sync.
            for a, bnd in split_cols:
                nc.sync.dma_start(in_sb[:, a:bnd], in_ap[:, a:bnd])

            ptiles = [[pp.tile([128, BANK], f32, tag="bank", name=f"pt{m}_{i}")
                       for i in range(NBANK)] for m in range(2)]
            out_sb = op.tile([128, 2 * COLS], bf, name="os")
            # cols written so far per bank (has_written high-water mark);
            # -1 = bank untouched.  Per-element has_written semantics make
            # first-write overwrite, later writes add.  Both c-halves run
            # inside the chunk loop so the tensor engine tracks the B stream
            # instead of queueing half the work behind the last input split.
            whi = [[-1] * NBANK for _ in range(2)]
            for k in korder:
                lo, hi = LOc[k], HIc[k]
                fo, bo = ft_off[k], b_off[k]
                for m in range(2):
                    lhsT = in_sb[:, fo + m * 128: fo + (m + 1) * 128]
                    for bk in range(lo // BANK, (hi - 1) // BANK + 1):
                        s = max(lo, bk * BANK)
                        e = min(hi, (bk + 1) * BANK)
                        nc.tensor.matmul(
                            ptiles[m][bk][:, s - bk * BANK: e - bk * BANK],
                            lhsT=lhsT,
                            rhs=in_sb[:, bo + s - lo: bo + e - lo],
                            start=whi[m][bk] < 0,
                            stop=k == last_k[bk],
                        )
                        whi[m][bk] = max(whi[m][bk], e)
            # evacuate each PSUM bank as it retires: half0 on vector, half1
            # on scalar (gpsimd cannot access PSUM); BOTH bank-3 copies on
            # vector (it frees first after the last matmul).  Three store
            # pieces — banks 0-1, bank 2, bank 3 — each one contiguous DMA
            # over both interleaved halves, triggered from sync (idle after
            # the input triggers) as soon as its banks' copies land; the
            # final piece is only bank 3 (~0.12MB).
            for bk in range(NBANK):
                w = min(BANK, COLS - bk * BANK)
                base = 2 * bk * BANK
                for m in range(2):
                    dst = out_sb[:, base + m * w: base + (m + 1) * w]
                    if m == 0 or bk == NBANK - 1:
                        nc.vector.tensor_copy(dst, ptiles[m][bk][:, :w])
                    else:
                        nc.scalar.copy(dst, ptiles[m][bk][:, :w])
                if bk >= 1:
                    a = 0 if bk == 1 else 2 * bk * BANK
                    e = min(2 * (bk + 1) * BANK, 2 * COLS)
                    nc.sync.dma_start(out_ap[:, a:e], out_sb[:, a:e])

    nc.compile()
    return nc


def _ensure_ntff_hook():
    """Some images lack antenv.axon_hooks; recreate it so a BASS_TRACE=1
    environment degrades to (or succeeds at) profiling instead of crashing."""
    import sys
    try:
        import antenv.axon_hooks  # noqa: F401
        return
    except ImportError:
        pass
    try:
        import types
        import antenv
        mod = types.ModuleType("antenv.axon_hooks")
        _hook = [None]
        mod.set_axon_ntff_profile_hook = lambda h: _hook.__setitem__(0, h)
        mod.get_axon_ntff_profile_hook = lambda: _hook[0]
        sys.modules["antenv.axon_hooks"] = mod
        antenv.axon_hooks = mod
        from trn_agent_boot.trn_boot import _ntff_profile_via_ctypes
        mod.set_axon_ntff_profile_hook(
            _ntff_profile_via_ctypes("/opt/axon/libaxon_pjrt.so"))
    except Exception:
        pass


def _unpack(res, hp):
    """out_core[c_chan, col(slot,q)] -> final[r, c_chan, p, q]"""
    COLS, NBANK = hp["COLS"], hp["NBANK"]
    final = np.zeros((hp["R"], C, POOLED, POOLED), dtype=np.float32)
    for c in range(NCORES):
        raw = np.asarray(res.results[c]["out"]).astype(np.float32)  # [128, 2C]
        out = np.zeros((C, COLS), dtype=np.float32)
        for bk in range(NBANK):
            w = min(BANK, COLS - bk * BANK)
            base = 2 * bk * BANK
            for m in range(2):
                out[m * 128:(m + 1) * 128, bk * BANK: bk * BANK + w] = (
                    raw[:, base + m * w: base + (m + 1) * w])
        gs = hp["groups"][c]
        if not gs:
            continue
        rgs = np.array([g[2] for g in gs])
        ps = np.array([g[3] for g in gs])
        sl = np.asarray(hp["slots"][c], dtype=np.int64)
        cols = out.reshape(C, -1, POOLED)[:, sl, :]     # [C, ngrp, 7]
        final[rgs, :, ps, :] = cols.transpose(1, 0, 2)
    return final


def kernel(features, rois):
    global LAST_RESULTS
    _ensure_ntff_hook()
    from concourse import bass_utils

    features = np.asarray(features, dtype=np.float32)
    rois = np.asarray(rois, dtype=np.float32)
    hp = _host_prep(features, rois)

    shape_key = (hp["NB"], hp["COLS"], hp["NBANK"],
                 tuple(hp["LOc"]), tuple(hp["HIc"]),
                 tuple(hp["korder"]),
                 tuple(sorted(hp["ft_off"].items())),
                 tuple(sorted(hp["b_off"].items())),
                 tuple(hp["split_cols"]),
                 int(hp["TOT"]))
    nc = _kernel_cache.get(shape_key)
    if nc is None:
        nc = _build_bass(shape_key)
        _kernel_cache[shape_key] = nc

    in_maps = [{"inp": np.ascontiguousarray(hp["IN"][c])}
               for c in range(NCORES)]
    # flaky-device insurance: a wedged core occasionally returns NaN — rerun.
    final = None
    for attempt in range(3):
        res = bass_utils.run_bass_kernel_spmd(nc, in_maps,
                                              core_ids=list(range(NCORES)))
        LAST_RESULTS = res
        final = _unpack(res, hp)
        if np.isfinite(final).all():
            break
    return final
